# revision 16
# baseline (speedup 1.0000x reference)
"""Multi-head causal attention with RoPE for TRN2, 8 NeuronCores.

Problem: B=2, T=2048, D=2048, 16 heads x head_dim 128, fp32 in/out.
  qkv = x @ Wqkv.T + bqkv ; RoPE(q,k) interleaved-pairs; causal softmax attention;
  out = attn_out @ Wo.T + bo.

Sharding: core c in 0..7 -> (batch b = c//4, head-group g = c%4 of 4 heads).
Each core computes its batch's partial output (its 4 heads' contribution through
the out-projection); host sums the 4 group partials per batch and adds bo.

v2 design (vs the fp32r DRAM-roundtrip baseline):
  - bf16 datapath everywhere on PE (weights, x, q, k, v, pt, O, Wo); fp32 PSUM.
  - No DRAM roundtrips: q/k/v drain from PSUM directly into SBUF in the exact
    layout attention needs (q tiles ARE [head, 512-q-tile] blocks; v tiles ARE
    [128 kpos, kc, head*128..] slices of the projection drain).
  - Phase A (projection) and attention pipelined per 512-token t-block:
    causal q-tile j only needs k,v up to (j+1)*512, i.e. t-blocks <= j.
    A-unit matmuls are drip-fed into attention's PE idle slots (attention alone
    is ACT(exp)-bound; PE has ~200ns/chunk spare).
  - RoPE rotate-half via partition-offset DVE muls (no PE perm matmul).
  - Softmax denominator for free: v is stored with a ones-column appended
    ([128, kc, 129]); attention output is computed TRANSPOSED per 128-wide
    q-sub-block (stationary = pt slice, moving = v_aug) so each PV matmul
    emits [q, 128 d + denom] in one pass. Normalization is then a per-partition
    tensor_scalar_mul with reciprocal_approx_fast on [128,4] (cheap), and the
    [q,d] -> [d,q] flip for the out-projection is 4 tiny PE transposes.
"""
import os
import sys

for _p in ("/opt/trn_rl_repo", "/root/.axon_site/_ro/trn_rl_repo"):
    if os.path.isdir(_p) and _p not in sys.path:
        sys.path.insert(0, _p)

import numpy as np
import ml_dtypes

import concourse.bacc as bacc
import concourse.mybir as mybir
import concourse.tile as tile
from concourse.bass_utils import run_bass_kernel_spmd

dt = mybir.dt
AF = mybir.ActivationFunctionType
BF16 = ml_dtypes.bfloat16

B = 2
T = 2048
D = 2048
NH = 16
HD = 128
ROPE_BASE = 10000.0
N_CORES = 8
GROUPS = 4          # head-groups (tensor-parallel axis)
HPG = NH // GROUPS  # heads per group = 4
QT = 512            # q-tile width in attention
NQT = T // QT       # 4
NCC = D // 128      # 16 contraction chunks
TB = 512            # phase-A t-block == QT
NTB = T // TB       # 4
SCALE = 1.0 / float(np.sqrt(HD))


def build(loop=1):
    """Emit the per-core BIR program (identical for all 8 cores)."""
    import contextlib

    nc = bacc.Bacc("TRN2", target_bir_lowering=False, debug=False)

    xp_d = nc.dram_tensor("xpack", [128, NCC, T], dt.bfloat16, kind="ExternalInput")
    wqp_d = nc.dram_tensor("wqpack", [8, 128, NCC * 128], dt.bfloat16,
                           kind="ExternalInput")
    wvp_d = nc.dram_tensor("wvpack", [128, NCC * 512], dt.bfloat16,
                           kind="ExternalInput")
    woT_d = nc.dram_tensor("woT", [HPG * HD, D], dt.bfloat16, kind="ExternalInput")
    cos_d = nc.dram_tensor("cosT", [HD, T], dt.float16, kind="ExternalInput")
    sin_d = nc.dram_tensor("sinT", [HD, T], dt.float16, kind="ExternalInput")
    mask_d = nc.dram_tensor("mask", [128, 128], dt.float32, kind="ExternalInput")
    bqk_d = nc.dram_tensor("bqk", [8 * 128, 1], dt.float32, kind="ExternalInput")
    bv_d = nc.dram_tensor("bvb", [HD, 512], dt.float32, kind="ExternalInput")
    idn_d = nc.dram_tensor("idn", [128, 128], dt.bfloat16, kind="ExternalInput")
    out_d = nc.dram_tensor("outp", [T, D], dt.bfloat16, kind="ExternalOutput")

    with tile.TileContext(nc, pool_alloc_mode="queue") as tc:
        with contextlib.ExitStack() as ctx:
            P = lambda *a, **kw: ctx.enter_context(tc.tile_pool(*a, **kw))
            kres = P(name="kres", bufs=1)
            wpool = P(name="wq", bufs=1)
            xpool = P(name="xb", bufs=1)
            s1p = P(name="s1p", bufs=2)
            rotp = P(name="rotp", bufs=2)
            csts = P(name="csts", bufs=1)
            qres = P(name="qres", bufs=1)
            vres = P(name="vres", bufs=1)
            ptp = P(name="ptp", bufs=4)
            osbp = P(name="osb", bufs=4)
            ohp = P(name="ohp", bufs=2)
            rcpp = P(name="rcpp", bufs=2)
            cdrp = P(name="cdr", bufs=2)
            aps = P(name="aps", bufs=2, space="PSUM")
            ps_s = P(name="ps_s", bufs=2, space="PSUM")
            poaA = P(name="poaA", bufs=1, space="PSUM")
            poaB = P(name="poaB", bufs=1, space="PSUM")
            ptrp = P(name="ptr", bufs=1, space="PSUM")
            cps = P(name="cps", bufs=1, space="PSUM")
            # ---- loop-invariant consts + persistent allocs (outside For_i)
            bqk_sb = csts.tile([128, 8, 1], dt.float32)
            nc.scalar.dma_start(
                out=bqk_sb, in_=bqk_d.ap().rearrange("(f p) o -> p f o", p=128)
            )
            cos_t = csts.tile([HD, T], dt.float16)
            sin_t = csts.tile([HD, T], dt.float16)
            nc.scalar.dma_start(out=cos_t, in_=cos_d.ap())
            nc.scalar.dma_start(out=sin_t, in_=sin_d.ap())
            bv_sb = csts.tile([HD, 512], dt.float32)
            nc.scalar.dma_start(out=bv_sb, in_=bv_d.ap())
            mask_t = csts.tile([128, 128], dt.float32)
            nc.scalar.dma_start(out=mask_t, in_=mask_d.ap())
            idn_t = csts.tile([128, 128], dt.bfloat16)
            nc.scalar.dma_start(out=idn_t, in_=idn_d.ap())

            k_rs = [kres.tile([HD, T], dt.bfloat16, tag=f"kr{h}", name=f"kr_{h}")
                    for h in range(HPG)]
            q_ts = {}
            for tb in range(NTB):
                for h in range(HPG):
                    q_ts[(tb, h)] = qres.tile([HD, QT], dt.bfloat16,
                                              tag=f"q{tb}_{h}", name=f"q_{tb}_{h}")
            v_aug = [vres.tile([128, 4 * NTB, 129], dt.bfloat16, tag=f"v{h}",
                               name=f"v_{h}") for h in range(HPG)]
            for h in range(HPG):
                nc.gpsimd.memset(v_aug[h][:, :, 128], 1.0)

            if loop > 1:
                ctx.enter_context(tc.For_i(0, loop, 1))

            # ---- per-iteration weight loads: sync queue (iteration-head
            # critical path; no tail work lives on sync so next iteration's
            # loads issue mid-previous-iteration as soon as readers free)
            wq_blocks = [None] * 8
            def load_wq(f):
                wq_b = wpool.tile([128, NCC, 128], dt.bfloat16, tag=f"wq{f}",
                                  name=f"wq_{f}")
                nc.sync.dma_start(
                    out=wq_b,
                    in_=wqp_d.ap()[f].rearrange("p (cc f) -> p cc f", f=128),
                )
                wq_blocks[f] = wq_b

            # ---------------- x slab loads (sync HWDGE) ---------------------
            x_slabs = {}

            def load_x(tb):
                tsl = slice(tb * TB, (tb + 1) * TB)
                if tb == 0:
                    parts = []
                    for qr in range(4):
                        xq = xpool.tile([128, 4, TB], dt.bfloat16, tag=f"x0q{qr}",
                                        name=f"x0_{qr}", bufs=1)
                        nc.sync.dma_start(
                            out=xq, in_=xp_d.ap()[:, 4 * qr:4 * (qr + 1), tsl]
                        )
                        parts.append(xq)
                    x_slabs[0] = ("quads", parts)
                else:
                    xs = xpool.tile([128, NCC, TB], dt.bfloat16, tag="xslab",
                                    name=f"x_{tb}")
                    nc.gpsimd.dma_start(out=xs, in_=xp_d.ap()[:, :, tsl])
                    x_slabs[tb] = ("slab", xs)

            def x_chunk(tb, cc):
                kind, v = x_slabs[tb]
                if kind == "quads":
                    return v[cc // 4][:, cc % 4, :]
                return v[:, cc, :]

            # ---------------- phase-A unit (drip-fed into PE stream) --------
            class AUnit:
                """One projection unit: 16 accumulating matmuls + drain.

                kind 'qk': f-block f (0-3 = q heads, 4-7 = k heads), out RoPE'd.
                kind 'v': ts4 sub-block, out v_aug slices (+bias).
                """

                def __init__(self, tb, kind, idx):
                    self.tb, self.kind, self.idx = tb, kind, idx
                    self.cc = 0
                    self.ps = None

                def step(self):
                    """Emit one PE matmul; returns True when unit is done."""
                    tb, kind, idx = self.tb, self.kind, self.idx
                    if self.ps is None:
                        self.ps = aps.tile(
                            [128, TB], dt.float32, tag="aps",
                            name=f"aps_{tb}_{kind}{idx}",
                        )
                    cc = self.cc
                    if kind == "qk":
                        nc.tensor.matmul(
                            self.ps, wq_blocks[idx][:, cc, :], x_chunk(tb, cc),
                            start=(cc == 0), stop=(cc == NCC - 1),
                        )
                    else:
                        nc.tensor.matmul(
                            self.ps,
                            x_chunk(tb, cc)[:, idx * 128:(idx + 1) * 128],
                            wv_b[:, cc, :],
                            start=(cc == 0), stop=(cc == NCC - 1),
                        )
                    self.cc += 1
                    if self.cc < NCC:
                        return False
                    self._drain()
                    return True

                def _drain(self):
                    tb, kind, idx = self.tb, self.kind, self.idx
                    tsl = slice(tb * TB, (tb + 1) * TB)
                    if kind == "qk":
                        f = idx
                        s1 = s1p.tile([128, TB], dt.bfloat16, tag="s1")
                        nc.vector.tensor_scalar_add(s1, self.ps, bqk_sb[:, f, :])
                        rot = rotp.tile([128, TB], dt.bfloat16, tag="rot")
                        # sin table halves are pre-swapped host-side so both
                        # SBUF inputs share a base partition (HW constraint)
                        nc.vector.tensor_mul(
                            out=rot[0:64, :], in0=s1[64:128, :],
                            in1=sin_t[64:128, tsl],
                        )
                        nc.vector.tensor_mul(
                            out=rot[64:128, :], in0=s1[0:64, :],
                            in1=sin_t[0:64, tsl],
                        )
                        nc.vector.tensor_mul(out=s1, in0=s1, in1=cos_t[:, tsl])
                        if f < 4:
                            nc.vector.tensor_add(out=q_ts[(tb, f)], in0=s1, in1=rot)
                        else:
                            nc.vector.tensor_add(
                                out=k_rs[f - 4][:, tsl], in0=s1, in1=rot
                            )
                    else:
                        kc = 4 * tb + idx
                        for h in range(HPG):
                            nc.vector.tensor_add(
                                out=v_aug[h][:, kc, 0:128],
                                in0=self.ps[:, h * 128:(h + 1) * 128],
                                in1=bv_sb[:, h * 128:(h + 1) * 128],
                            )

            # ---------------- cproj unit (out-projection, drip-fed) ---------
            class CUnit:
                """Out-projection for q-tile pj, row-block tt: 4 oo groups of
                4 accumulating matmuls, drained to one [128, 4*512] bf16 tile,
                then one DMA."""

                def __init__(self, pj, tt, o_heads):
                    self.pj, self.tt, self.o_heads = pj, tt, o_heads
                    self.step_i = 0
                    self.ps = None
                    self.dr = None

                def step(self):
                    pj, tt = self.pj, self.tt
                    oo, h = divmod(self.step_i, HPG)
                    if h == 0:
                        self.ps = cps.tile(
                            [128, QT], dt.float32, tag="cps",
                            name=f"cps_{pj}_{tt}_{oo}",
                        )
                        if oo == 0:
                            self.dr = cdrp.tile(
                                [128, 4, QT], dt.bfloat16, tag="cdr",
                                name=f"cdr_{pj}_{tt}",
                            )
                    nc.tensor.matmul(
                        self.ps,
                        self.o_heads[h][:, tt, :],
                        wo_sb[:, h, oo * QT:(oo + 1) * QT],
                        start=(h == 0), stop=(h == HPG - 1),
                    )
                    self.step_i += 1
                    if h == HPG - 1:
                        eng = nc.scalar if (oo % 2 == 0) else nc.vector
                        if oo % 2 == 0:
                            nc.scalar.copy(out=self.dr[:, oo, :], in_=self.ps)
                        else:
                            nc.vector.tensor_copy(out=self.dr[:, oo, :], in_=self.ps)
                        if oo == D // QT - 1:
                            nc.gpsimd.dma_start(
                                out=out_d.ap()[
                                    pj * QT + tt * 128: pj * QT + (tt + 1) * 128, :
                                ].rearrange("p (oo f) -> p oo f", f=QT),
                                in_=self.dr,
                            )
                            return True
                    return False

            # ---------------- filler scheduler ------------------------------
            from collections import deque
            work = deque()

            def fill(n):
                for _ in range(n):
                    if not work:
                        return
                    if work[0].step():
                        work.popleft()

            def flush_A(tb):
                # emit everything still pending up to and including A(tb) units
                while any(isinstance(u, AUnit) and u.tb <= tb for u in work):
                    if work[0].step():
                        work.popleft()

            def queue_A(tb):
                for f in (0, 4, 1, 5, 2, 6, 3, 7):
                    work.append(AUnit(tb, "qk", f))
                for ts4 in range(4):
                    work.append(AUnit(tb, "v", ts4))

            # ---------------- attention head --------------------------------
            def att_head(j, h, o_heads):
                q_t = q_ts[(j, h)]
                nkc = 4 * (j + 1)

                def col0(kc):
                    m = kc - 4 * j
                    return 0 if m <= 0 else 128 * m

                def s_matmul(kc):
                    c0 = col0(kc)
                    psum_s = ps_s.tile(
                        [128, QT], dt.float32, name=f"s_{j}_{h}_{kc}", tag="psum_s",
                    )
                    nc.tensor.matmul(
                        psum_s[:, c0:],
                        k_rs[h][:, kc * 128:(kc + 1) * 128],
                        q_t[:, c0:],
                        start=True, stop=True,
                    )
                    return psum_s

                # S + exp for all chunks; pt tiles persist across the head
                pts = []
                s_next = s_matmul(0)
                for kc in range(nkc):
                    psum_s = s_next
                    if kc + 1 < nkc:
                        s_next = s_matmul(kc + 1)
                    fill(1)
                    c0 = col0(kc)
                    m = kc - 4 * j
                    pt = ptp.tile([128, QT], dt.bfloat16, tag="pt", bufs=16,
                                  name=f"pt_{j}_{h}_{kc}")
                    nc.scalar.activation(
                        out=pt[:, c0:], in_=psum_s[:, c0:], func=AF.Exp,
                        scale=SCALE,
                    )
                    if m >= 0:
                        nc.vector.tensor_mul(
                            out=pt[:, c0:c0 + 128], in0=pt[:, c0:c0 + 128],
                            in1=mask_t,
                        )
                    pts.append(pt)

                # one accumulation group per PSUM bank (A/B ping-pong): an
                # interleaved second group's start corrupts the first in-bank
                rcp = rcpp.tile([128, 4], dt.float32, tag="rcp")
                o_sb = osbp.tile([128, 4, 128], dt.bfloat16, tag="osb")
                for s in range(4):
                    pool = poaA if s % 2 == 0 else poaB
                    oa = pool.tile([128, 129], dt.float32,
                                   tag=f"oa{'AB'[s % 2]}",
                                   name=f"oa_{j}_{h}_{s}")
                    last = 4 * j + s
                    for kc in range(last + 1):
                        nc.tensor.matmul(
                            oa,
                            pts[kc][:, s * 128:(s + 1) * 128],
                            v_aug[h][:, kc, :],
                            start=(kc == 0), stop=(kc == last),
                        )
                        if kc % 2 == 0:
                            fill(1)
                    nc.vector.reciprocal_approx_fast(
                        rcp[:, s:s + 1], oa[:, 128:129]
                    )
                    nc.vector.tensor_scalar_mul(
                        o_sb[:, s, :], oa[:, 0:128], rcp[:, s:s + 1]
                    )
                work.append(TransUnit(j, h, o_sb, o_heads))

            class TransUnit:
                """Deferred [q,d] -> [d,q] flip of a head's normalized output:
                4 PE transposes + 1 ACT copy, drip-fed as filler."""

                def __init__(self, j, h, o_sb, o_heads):
                    self.j, self.h, self.o_sb, self.o_heads = j, h, o_sb, o_heads
                    self.s = 0
                    self.ptr = None

                def step(self):
                    if self.ptr is None:
                        self.ptr = ptrp.tile([128, 4, 128], dt.bfloat16,
                                             tag="ptr",
                                             name=f"ptr_{self.j}_{self.h}")
                    s = self.s
                    nc.tensor.matmul(
                        self.ptr[:, s, :], self.o_sb[:, s, :], idn_t,
                        is_transpose=True,
                    )
                    self.s += 1
                    if self.s < 4:
                        return False
                    o_h = ohp.tile([128, 4, 128], dt.bfloat16, tag=f"oh{self.h}",
                                   name=f"oh_{self.j}_{self.h}")
                    nc.scalar.copy(out=o_h, in_=self.ptr)
                    self.o_heads[self.h] = o_h
                    return True

            # ---------------- main schedule ---------------------------------
            load_x(0)
            for f in (0, 4, 1, 5, 2, 6, 3, 7):
                load_wq(f)
            wv_b = wpool.tile([128, NCC, 512], dt.bfloat16)
            nc.sync.dma_start(
                out=wv_b, in_=wvp_d.ap().rearrange("p (cc f) -> p cc f", f=512),
            )
            wo_sb = wpool.tile([128, HPG, D], dt.bfloat16)
            nc.sync.dma_start(
                out=wo_sb, in_=woT_d.ap().rearrange("(hh p) o -> p hh o", p=128)
            )
            queue_A(0)
            flush_A(0)

            prev_o = None
            for j in range(NQT):
                if j + 1 < NTB:
                    load_x(j + 1)
                    queue_A(j + 1)
                o_heads = [None] * HPG
                for h in range(HPG):
                    att_head(j, h, o_heads)
                    if prev_o is not None:
                        work.append(CUnit(j - 1, h, prev_o))
                if j + 1 < NTB:
                    flush_A(j + 1)
                prev_o = o_heads
            # flush all remaining deferred work (incl. TransUnits of j=3)
            # BEFORE the tail out-projection reads o_heads
            fill(10 ** 9)
            # tail: out-projection of the last q-tile
            for tt in range(4):
                u = CUnit(NQT - 1, tt, prev_o)
                while not u.step():
                    pass
    nc.compile()
    return nc


# ---------------------------------------------------------------------------
# Host side
# ---------------------------------------------------------------------------

_DEINT = np.concatenate([np.arange(0, HD, 2), np.arange(1, HD, 2)])  # de-interleave


def _rope_tables():
    half = HD // 2
    inv_freq = 1.0 / (ROPE_BASE ** (np.arange(half, dtype=np.float64) / half))
    t = np.arange(T, dtype=np.float64)
    fr = t[None, :] * inv_freq[:, None]          # (64, T)
    cos = np.concatenate([np.cos(fr), np.cos(fr)], axis=0).astype(np.float16)
    # halves swapped: row i<64 holds +sin(f_i) (used for rot[64+i]),
    # row 64+i holds -sin(f_i) (used for rot[i])
    sin = np.concatenate([np.sin(fr), -np.sin(fr)], axis=0).astype(np.float16)
    return cos, sin


def _mask():
    kk = np.arange(128)[:, None]
    qq = np.arange(128)[None, :]
    return (kk <= qq).astype(np.float32)


def make_in_maps(x, Wqkv, bqkv, Wo, bo):
    cos, sin = _rope_tables()
    mask = _mask()
    idn = np.eye(128, dtype=BF16)

    Wq = Wqkv[0 * D:1 * D]
    Wk = Wqkv[1 * D:2 * D]
    Wv = Wqkv[2 * D:3 * D]
    bq = bqkv[0 * D:1 * D]
    bk = bqkv[1 * D:2 * D]
    bv = bqkv[2 * D:3 * D]

    in_maps = []
    for c in range(N_CORES):
        b, g = divmod(c, GROUPS)
        hsl = slice(g * HPG * HD, (g + 1) * HPG * HD)
        # de-interleaved row order for q,k heads of this group
        rows = np.arange(g * HPG * HD, (g + 1) * HPG * HD).reshape(HPG, HD)
        rows = rows[:, _DEINT].reshape(-1)

        wq = Wq[rows]                       # (512, D)
        wk = Wk[rows]
        wv = Wv[hsl]                        # natural order
        wqkT = np.concatenate([wq, wk], axis=0).T  # (D, 1024)
        wqpack = np.ascontiguousarray(
            wqkT.reshape(NCC, 128, 8, 128)      # (cc, p, fb, f)
                .transpose(2, 1, 0, 3)           # (fb, p, cc, f)
                .reshape(8, 128, NCC * 128)
        ).astype(BF16)
        wvT = wv.T                               # (D, 512)
        wvpack = np.ascontiguousarray(
            wvT.reshape(NCC, 128, 512).transpose(1, 0, 2).reshape(128, NCC * 512)
        ).astype(BF16)
        woT = np.ascontiguousarray(Wo[:, hsl].T).astype(BF16)  # (512, D)

        bqk = np.concatenate([bq[rows], bk[rows]]).astype(np.float32)[:, None]
        bvb = np.broadcast_to(bv[hsl].astype(np.float32), (HD, 512)).copy()

        xb = np.asarray(x[b]).astype(BF16)       # (T, D) -> pack (p, cc, t)
        xpack = np.ascontiguousarray(xb.T.reshape(NCC, 128, T).transpose(1, 0, 2))

        in_maps.append({
            "xpack": xpack,
            "wqpack": wqpack,
            "wvpack": wvpack,
            "woT": woT,
            "cosT": cos,
            "sinT": sin,
            "mask": mask,
            "bqk": bqk,
            "bvb": bvb,
            "idn": idn,
        })
    return in_maps


_NC_CACHE = {}


def _get_nc(loop=1):
    if loop not in _NC_CACHE:
        _NC_CACHE[loop] = build(loop=loop)
    return _NC_CACHE[loop]


def kernel(x, Wqkv, bqkv, Wo, bo):
    x = np.asarray(x)
    Wqkv = np.asarray(Wqkv)
    bqkv = np.asarray(bqkv)
    Wo = np.asarray(Wo)
    bo = np.asarray(bo)

    nc = _get_nc()
    in_maps = make_in_maps(x, Wqkv, bqkv, Wo, bo)
    res = run_bass_kernel_spmd(nc, in_maps, core_ids=list(range(N_CORES)))

    out = np.zeros((B, T, D), dtype=np.float32)
    for c in range(N_CORES):
        b = c // GROUPS
        out[b] += res.results[c]["outp"].astype(np.float32)
    out += bo.astype(np.float32)[None, None, :]
    return out
